# revision 30
# baseline (speedup 1.0000x reference)
"""DPS perturbed-top-k patch-extraction kernel for Trainium2 (Bass/Tile), v5.

Contract: kernel(**inputs) takes the FULL inputs
    x_high  (8, 3, 512, 512) f32
    scores_2d (8, 16, 16) f32
    noise   (8, 500, 256) f32
and returns the FULL output (128, 3, 64, 64) f32.

Sharding: pure data-parallel over batch b across the 8 NeuronCores.
The per-core input layout transform (pad + 32x32 block gather into the
three a2-major B operand matrices, bf16) happens on the host during
sharding, mirroring the host-side output unscramble.  The device kernel
reads only 2.5 MB: B (3 x 108 x 3072 bf16), noise (f32), scores.

Device pipeline (everything gated by the indicator chain):
  * pert = s_row + SIG*noise via PE matmuls into PSUM (4 n-chunks).
  * top-16 threshold per sample: DVE max8 / match_replace / max8.
  * A' = Sign(pert - t16 + eps) on ACT (bf16, +-1).
  * prefix over d on PE: transpose A' then triangular-ones matmuls
    -> cntT' = 2*cnt - (d+1) in PSUM; DVE adds (d-1) -> W = 2*(cnt-1)
    packed bf16 in SBUF.
  * G_k(d) = #{n: cnt >= k+1} via per-k accumulations on W, split over
    three engines (DVE is_ge k<KG, GPSIMD is_ge KG<=k<KD, ACT Sign-sum
    k>=KD) and two n-phases so counting starts after chunk 1.
  * indicators from G differences; INDr[m] = shifted linear slices
    (a2-major layout) transposed on PE.
  * main matmul: out[64, 3072] = sum_m INDr[m]^T @ B[m] in 6 chunks of
    512 cols, m-outer, two PSUM waves; bf16 output, host upcasts.
"""
import numpy as np
from contextlib import ExitStack

# ---- problem constants (hardcoded per spec) ----
NB = 8
C = 3
H = W = 512
GS = 16
GE = 18          # embedded grid stride (d' = 18i + j)
D2 = 256
D3 = GE * GE     # 324
K = 16
N = 500
NCH = 4
NP = 125
CM = 108         # B partitions per tile (6 a2 x 18 b, a2-major)
PATCH = 64
BLK = 32
SIG = 0.05
INV_N = 1.0 / 500.0
EPS = 1e-7
F = C * BLK * BLK      # 3072 elems per block partition
KG = 9                 # k in [0, KG): DVE is_ge counts
KD = 9                # k in [KG, KD): GPSIMD is_ge; [KD, 16): ACT Sign

_CACHE = {}


def _build_nc():
    import concourse.bacc as bacc
    import concourse.bass as bass
    import concourse.mybir as mybir
    import concourse.tile as tile

    F32 = mybir.dt.float32
    BF16 = mybir.dt.bfloat16
    ALU = mybir.AluOpType
    ACTF = mybir.ActivationFunctionType
    AP = bass.AP

    nc = bacc.Bacc("TRN2", target_bir_lowering=False, debug=False)
    bx_d = nc.dram_tensor("bx", (3 * CM * F,), BF16, kind="ExternalInput")
    sc_d = nc.dram_tensor("sc", (GS, GS), F32, kind="ExternalInput")
    nz_d = nc.dram_tensor("nz", (N, D2), F32, kind="ExternalInput")
    o_d = nc.dram_tensor("o", (64, F), BF16, kind="ExternalOutput")

    with tile.TileContext(nc) as tc, ExitStack() as ctx:
        sb = ctx.enter_context(tc.tile_pool(name="sb", bufs=1))
        ps = ctx.enter_context(tc.tile_pool(name="ps", bufs=1, space="PSUM"))

        def ap_of(t, off_elems, dims):
            return AP(t.tensor, t[:].offset + off_elems, dims)

        dma_s = nc.sync.dma_start
        dma_a = nc.scalar.dma_start

        # ---------------- loads -----------------------------------------
        # sync ring: scores + noise (the critical chain); scalar ring: B
        s256 = sb.tile([1, D2], F32)
        dma_a(s256[:], sc_d[:].rearrange("a b -> (a b)").unsqueeze(0))
        # noise: partition n holds chunks (n, n+125, n+250, n+375) side by
        # side; 2 DMAs with contiguous 2KB rows (descriptor-cost bound)
        nz_sb = sb.tile([128, 4 * D2], F32)
        for j in range(2):
            dma_s(ap_of(nz_sb, 2 * D2 * j, [[4 * D2, NP], [1, 2 * D2]]),
                  AP(nz_d, 2 * j * NP * D2,
                     [[D2, NP], [NP * D2, 2], [1, D2]]))
        # B: one tile [108, 3*F], host-interleaved (p, m, f); single DMA
        # behind noise on the same (sync) ring -- FIFO keeps noise first
        B_all = sb.tile([CM, 3 * F], BF16)
        dma_s(ap_of(B_all, 0, [[3 * F, CM], [1, 3 * F]]),
              AP(bx_d, 0, [[3 * F, CM], [1, 3 * F]]))
        B = [ap_of(B_all, m * F, [[3 * F, CM], [1, F]]) for m in range(3)]

        # ---------------- scores normalization (DVE) --------------------
        smax = sb.tile([1, 1], F32)
        smin = sb.tile([1, 1], F32)
        Dt = sb.tile([1, 1], F32)
        rD = sb.tile([1, 1], F32)
        s_row = sb.tile([1, D2], F32)
        with tc.high_priority():
            nc.vector.tensor_reduce(smax[:], s256[:],
                                    axis=mybir.AxisListType.X, op=ALU.max)
            nc.vector.tensor_reduce(smin[:], s256[:],
                                    axis=mybir.AxisListType.X, op=ALU.min)
            nc.vector.tensor_scalar(Dt[:], smax[:], smin[:], 1e-5,
                                    op0=ALU.subtract, op1=ALU.add)
            nc.vector.reciprocal(rD[:], Dt[:])
            nc.vector.tensor_scalar(s_row[:], s256[:], smin[:], rD[:],
                                    op0=ALU.subtract, op1=ALU.mult)

        # ---------------- constants (f32 iota; i32 ops are ~10x slower) --
        iota_t = sb.tile([128, 128], F32)
        nc.gpsimd.iota(iota_t[:], pattern=[[-1, 128]], base=0,
                       channel_multiplier=1,
                       allow_small_or_imprecise_dtypes=True)  # p - j
        ident_f32 = sb.tile([128, 128], F32)
        nc.vector.tensor_scalar(ident_f32[:], iota_t[:], 0, None,
                                op0=ALU.is_equal)
        ident_bf = sb.tile([128, 128], BF16)
        nc.vector.tensor_scalar(ident_bf[:], iota_t[:], 0, None,
                                op0=ALU.is_equal)
        tri_bf = sb.tile([128, 128], BF16)
        nc.vector.tensor_scalar(tri_bf[:], iota_t[:], 0.5, None,
                                op0=ALU.is_le)  # [p <= j]
        ones_bf = sb.tile([128, 128], BF16)
        nc.vector.memset(ones_bf[:], 1.0)
        ones_f = sb.tile([1, 128], F32)
        nc.vector.memset(ones_f[:], 1.0)
        diag05 = sb.tile([128, 128], F32)
        nc.vector.tensor_scalar(diag05[:], iota_t[:], 0, SIG,
                                op0=ALU.is_equal, op1=ALU.mult)
        # ACT G-Sign bias col k = 0.5 - 2k
        iota_r = sb.tile([128, K], F32)
        nc.gpsimd.iota(iota_r[:], pattern=[[-1, K]], base=0,
                       channel_multiplier=0,
                       allow_small_or_imprecise_dtypes=True)  # -j
        bias_f = sb.tile([128, K], F32)
        nc.vector.tensor_scalar(bias_f[:], iota_r[:], 1.0, -0.5,
                                op0=ALU.mult, op1=ALU.add)

        # ---------------- per-chunk top-k chain --------------------------
        ApT_ps = [ps.tile([128, 512], BF16, tag="pp", name=f"ApT{u}",
                          bufs=2) for u in range(2)]
        ApT = [sb.tile([128, 512], BF16, name=f"ApTs{u}") for u in range(2)]
        for u in range(2):
            nc.gpsimd.memset(ApT[u][:], 0.0)
        cntT = [ps.tile([128, 512], F32, tag=f"ct{u}", name=f"cntT{u}")
                for u in range(2)]

        Apcs = []
        for c in range(NCH):
            pert_ps = ps.tile([128, D2], F32, tag="pt",
                              name=f"pertps{c}", bufs=2)
            nc.tensor.matmul(pert_ps[0:NP, :], ones_f[:, 0:NP], s_row[:],
                             start=True, stop=False)
            nc.tensor.matmul(pert_ps[0:NP, :], diag05[0:NP, 0:NP],
                             nz_sb[0:NP, D2 * c:D2 * (c + 1)],
                             start=False, stop=True)
            top8 = sb.tile([128, 8], F32, tag="top8", name=f"top8_{c}", bufs=2)
            nc.vector.max(top8[0:NP, :], pert_ps[0:NP, :])
            pert2 = sb.tile([128, D2], F32, tag="pert2", name=f"pert2_{c}",
                            bufs=2)
            nc.vector.match_replace(pert2[0:NP, :], top8[0:NP, :],
                                    pert_ps[0:NP, :], -1.0e30)
            top8b = sb.tile([128, 8], F32, tag="top8b", name=f"top8b_{c}",
                            bufs=2)
            nc.vector.max(top8b[0:NP, :], pert2[0:NP, :])
            Apc = sb.tile([128, D2], BF16, name=f"Ap{c}")
            nc.vector.tensor_scalar(Apc[0:NP, :], pert_ps[0:NP, :],
                                    top8b[0:NP, 7:8], None, op0=ALU.is_ge)
            Apcs.append(Apc)
        for c in range(NCH):
            cs = slice(128 * c, 128 * (c + 1))
            for u in range(2):
                nc.tensor.transpose(ApT_ps[u][:, 128 * c:128 * c + NP],
                                    Apcs[c][0:NP, 128 * u:128 * (u + 1)],
                                    ident_bf[0:NP, 0:NP])
                (nc.vector.tensor_copy if u == 0 else
                 (lambda d, s_: nc.scalar.copy(d, s_)))(
                    ApT[u][:, 128 * c:128 * c + NP],
                    ApT_ps[u][:, 128 * c:128 * c + NP])
            nc.tensor.matmul(cntT[1][:, cs], ones_bf[:], ApT[0][:, cs],
                             start=True, stop=False)
            nc.tensor.matmul(cntT[1][:, cs], tri_bf[:], ApT[1][:, cs],
                             start=False, stop=True)
            nc.tensor.matmul(cntT[0][:, cs], tri_bf[:], ApT[0][:, cs],
                             start=True, stop=True)

        # ---------------- G: threshold counts on W ----------------------
        # k < KD: DVE is_ge counts; k >= KD: ACT Sign-sums (2G - 512);
        # full-width [128, 512] ops (better fixed-overhead amortization)
        Gc = [sb.tile([128, K], F32, name=f"Gc{u}") for u in range(2)]
        Sa = [sb.tile([128, K], F32, name=f"Sa{u}") for u in range(2)]
        scr_v = sb.tile([128, 512], BF16, name="scr_v")
        scr_a = sb.tile([128, 512], BF16, name="scr_a")
        for u in range(2):
            for k in range(0, KD):
                nc.vector.tensor_scalar(
                    scr_v[:], cntT[u][:], float(k) + 0.5, None,
                    op0=ALU.is_ge, op1=ALU.add,
                    accum_out=Gc[u][:, k:k + 1])
            for k in range(KD, K):
                nc.scalar.activation(
                    scr_a[:], cntT[u][:], ACTF.Sign,
                    bias=bias_f[:, k:k + 1], scale=1.0,
                    accum_out=Sa[u][:, k:k + 1])
        for u in range(2):
            nc.vector.tensor_scalar(Gc[u][:, KD:K], Sa[u][:, KD:K], 0.5,
                                    None, op0=ALU.mult)

        # ---------------- gct -> compact indicator ----------------------
        gct_sb = sb.tile([16, 1 + D2], F32)
        # col0 = G-form at d=-1: 0 for is_ge rows, 0.5*(-512)*INV_N for Sign
        nc.vector.tensor_scalar(gct_sb[:, 0:1], iota_t[0:16, 0:1],
                                float(KD) - 0.5, -256.0 * INV_N,
                                op0=ALU.is_ge, op1=ALU.mult)
        for u in range(2):
            gct_ps = ps.tile([16, 128], F32, tag="pp", name=f"gct{u}", bufs=2)
            nc.tensor.transpose(gct_ps[:], Gc[u][:], ident_f32[:])
            nc.vector.tensor_scalar(gct_sb[:, 1 + 128 * u:1 + 128 * (u + 1)],
                                    gct_ps[:], INV_N, None, op0=ALU.mult)
        indC = sb.tile([16, D2], F32)
        nc.vector.tensor_tensor(indC[:], gct_sb[:, 1:1 + D2],
                                gct_sb[:, 0:D2], op=ALU.subtract)
        # embed into d' = 18i + j (rims stay zero)
        indT_pad = sb.tile([16, 19 + D3], F32)
        nc.vector.memset(indT_pad[:], 0.0)
        nc.vector.tensor_copy(
            ap_of(indT_pad, 19, [[19 + D3, 16], [GE, GS], [1, GS]]),
            ap_of(indC, 0, [[D2, 16], [GS, GS], [1, GS]]))

        # ---------------- INDr: linear slices (a2-major) ----------------
        INDr = [sb.tile([CM, 64], BF16, name=f"INDr{m}") for m in range(3)]
        for m in range(3):
            tg = "pp" if m == 2 else f"ct{m}"
            ind_ps = ps.tile([CM, 64], F32, tag=tg, name=f"indps{m}",
                             bufs=2 if m == 2 else None)
            for hq in range(2):
                for wq in range(2):
                    q = 2 * hq + wq
                    s = GE * hq + wq
                    tmp = sb.tile([16, CM], F32, tag=f"iperm{q % 2}",
                                  name=f"iperm{m}_{q}", bufs=2)
                    nc.vector.tensor_copy(
                        tmp[:],
                        indT_pad[:, 19 + CM * m - s:19 + CM * (m + 1) - s])
                    nc.tensor.transpose(ind_ps[:, 16 * q:16 * (q + 1)],
                                        tmp[:], ident_f32[0:16, 0:16])
            nc.vector.tensor_copy(INDr[m][:], ind_ps[:])

        # ---------------- main matmul (bf16) + output -------------------
        act_cp = lambda d, s_: nc.scalar.copy(d, s_)
        dve_cp = nc.vector.tensor_copy
        osb = sb.tile([64, F], BF16)
        for w in range(3):
            mm = [ps.tile([64, 512], F32, tag="mm", name=f"mm{2 * w + i}",
                          bufs=2) for i in range(2)]
            for m in range(3):
                for i in range(2):
                    t = 2 * w + i
                    nc.tensor.matmul(
                        mm[i][:], INDr[m][:],
                        ap_of(B_all, m * F + 512 * t, [[3 * F, CM], [1, 512]]),
                        start=(m == 0), stop=(m == 2))
            for i in range(2):
                t = 2 * w + i
                act_cp(osb[:, 512 * t:512 * (t + 1)], mm[i][:])
                dma_s(AP(o_d, 512 * t, [[F, 64], [1, 512]]),
                      ap_of(osb, 512 * t, [[F, 64], [1, 512]]))

    nc.compile()
    return nc


def _get_nc():
    if "nc" not in _CACHE:
        _CACHE["nc"] = _build_nc()
    return _CACHE["nc"]


def _host_bx(x):
    """x (3, 512, 512) f32 -> (3*108*3072,) bf16 a2-major block gather."""
    import ml_dtypes
    xp = np.zeros((C, 576, 576), np.float32)
    xp[:, 16:528, 16:528] = x
    blocks = xp.reshape(C, GE, BLK, GE, BLK)       # (c, a, h', b, w')
    b_all = blocks.transpose(1, 3, 2, 0, 4)        # (a, b, h', c, w')
    bm = b_all.reshape(3, 6 * GE, BLK * C * BLK)   # (m, p, f)
    bm = bm.transpose(1, 0, 2)                     # (p, m, f)
    return np.ascontiguousarray(bm).reshape(-1).astype(ml_dtypes.bfloat16)


def _unscramble(o2):
    # o2 (64, 3072) rows = (hq, wq, k), cols = (h', c, w')
    return (np.asarray(o2).astype(np.float32)
              .reshape(2, 2, K, 32, C, 32)
              .transpose(2, 4, 0, 3, 1, 5)
              .reshape(K, C, PATCH, PATCH))


def _run(x_high, scores_2d, noise, trace=False):
    from concourse import bass_utils
    nc = _get_nc()
    x_high = np.ascontiguousarray(x_high, dtype=np.float32)
    scores_2d = np.ascontiguousarray(scores_2d, dtype=np.float32)
    noise = np.ascontiguousarray(noise, dtype=np.float32)
    in_maps = [
        {"bx": _host_bx(x_high[i]), "sc": scores_2d[i], "nz": noise[i]}
        for i in range(NB)
    ]
    res = bass_utils.run_bass_kernel_spmd(
        nc, in_maps, core_ids=list(range(NB)), trace=trace)
    out = np.concatenate(
        [_unscramble(res.results[i]["o"])[None] for i in range(NB)],
        axis=0).reshape(NB * K, C, PATCH, PATCH)
    return out, res


def kernel(x_high, scores_2d, noise):
    out, _ = _run(x_high, scores_2d, noise, trace=False)
    return out


# revision 31
# speedup vs baseline: 1.0733x; 1.0733x over previous
"""DPS perturbed-top-k patch-extraction kernel for Trainium2 (Bass/Tile), v5.

Contract: kernel(**inputs) takes the FULL inputs
    x_high  (8, 3, 512, 512) f32
    scores_2d (8, 16, 16) f32
    noise   (8, 500, 256) f32
and returns the FULL output (128, 3, 64, 64) f32.

Sharding: pure data-parallel over batch b across the 8 NeuronCores.
The per-core input layout transform (pad + 32x32 block gather into the
three a2-major B operand matrices, bf16) happens on the host during
sharding, mirroring the host-side output unscramble.  The device kernel
reads only 2.5 MB: B (3 x 108 x 3072 bf16), noise (f32), scores.

Device pipeline (everything gated by the indicator chain):
  * pert = s_row + SIG*noise via PE matmuls into PSUM (4 n-chunks).
  * top-16 threshold per sample: DVE max8 / match_replace / max8.
  * A' = Sign(pert - t16 + eps) on ACT (bf16, +-1).
  * prefix over d on PE: transpose A' then triangular-ones matmuls
    -> cntT' = 2*cnt - (d+1) in PSUM; DVE adds (d-1) -> W = 2*(cnt-1)
    packed bf16 in SBUF.
  * G_k(d) = #{n: cnt >= k+1} via per-k accumulations on W, split over
    three engines (DVE is_ge k<KG, GPSIMD is_ge KG<=k<KD, ACT Sign-sum
    k>=KD) and two n-phases so counting starts after chunk 1.
  * indicators from G differences; INDr[m] = shifted linear slices
    (a2-major layout) transposed on PE.
  * main matmul: out[64, 3072] = sum_m INDr[m]^T @ B[m] in 6 chunks of
    512 cols, m-outer, two PSUM waves; bf16 output, host upcasts.
"""
import numpy as np
from contextlib import ExitStack

# ---- problem constants (hardcoded per spec) ----
NB = 8
C = 3
H = W = 512
GS = 16
GE = 18          # embedded grid stride (d' = 18i + j)
D2 = 256
D3 = GE * GE     # 324
K = 16
N = 500
NCH = 4
NP = 125
CM = 108         # B partitions per tile (6 a2 x 18 b, a2-major)
PATCH = 64
BLK = 32
SIG = 0.05
INV_N = 1.0 / 500.0
EPS = 1e-7
F = C * BLK * BLK      # 3072 elems per block partition
KG = 9                 # k in [0, KG): DVE is_ge counts
KD = 9                # k in [KG, KD): GPSIMD is_ge; [KD, 16): ACT Sign

_CACHE = {}


def _build_nc():
    import concourse.bacc as bacc
    import concourse.bass as bass
    import concourse.mybir as mybir
    import concourse.tile as tile

    F32 = mybir.dt.float32
    BF16 = mybir.dt.bfloat16
    ALU = mybir.AluOpType
    ACTF = mybir.ActivationFunctionType
    AP = bass.AP

    nc = bacc.Bacc("TRN2", target_bir_lowering=False, debug=False)
    bx_d = nc.dram_tensor("bx", (3 * CM * F,), BF16, kind="ExternalInput")
    sc_d = nc.dram_tensor("sc", (GS, GS), F32, kind="ExternalInput")
    nz_d = nc.dram_tensor("nz", (N, D2), F32, kind="ExternalInput")
    o_d = nc.dram_tensor("o", (64, F), BF16, kind="ExternalOutput")

    with tile.TileContext(nc) as tc, ExitStack() as ctx:
        sb = ctx.enter_context(tc.tile_pool(name="sb", bufs=1))
        ps = ctx.enter_context(tc.tile_pool(name="ps", bufs=1, space="PSUM"))

        def ap_of(t, off_elems, dims):
            return AP(t.tensor, t[:].offset + off_elems, dims)

        dma_s = nc.sync.dma_start
        dma_a = nc.scalar.dma_start

        # ---------------- loads -----------------------------------------
        # sync ring: scores + noise (the critical chain); scalar ring: B
        s256 = sb.tile([1, D2], F32)
        dma_a(s256[:], sc_d[:].rearrange("a b -> (a b)").unsqueeze(0))
        # noise: partition n holds chunks (n, n+125, n+250, n+375) side by
        # side; 2 DMAs with contiguous 2KB rows (descriptor-cost bound)
        nz_sb = sb.tile([128, 4 * D2], F32)
        for j in range(2):
            dma_s(ap_of(nz_sb, 2 * D2 * j, [[4 * D2, NP], [1, 2 * D2]]),
                  AP(nz_d, 2 * j * NP * D2,
                     [[D2, NP], [NP * D2, 2], [1, D2]]))
        # B: one tile [108, 3*F], host-interleaved (p, m, f); single DMA
        # behind noise on the same (sync) ring -- FIFO keeps noise first
        B_all = sb.tile([CM, 3 * F], BF16)
        dma_s(ap_of(B_all, 0, [[3 * F, CM], [1, 3 * F]]),
              AP(bx_d, 0, [[3 * F, CM], [1, 3 * F]]))
        B = [ap_of(B_all, m * F, [[3 * F, CM], [1, F]]) for m in range(3)]

        # ---------------- scores normalization (DVE) --------------------
        smax = sb.tile([1, 1], F32)
        smin = sb.tile([1, 1], F32)
        Dt = sb.tile([1, 1], F32)
        rD = sb.tile([1, 1], F32)
        s_row = sb.tile([1, D2], F32)
        with tc.high_priority():
            nc.vector.tensor_reduce(smax[:], s256[:],
                                    axis=mybir.AxisListType.X, op=ALU.max)
            nc.vector.tensor_reduce(smin[:], s256[:],
                                    axis=mybir.AxisListType.X, op=ALU.min)
            nc.vector.tensor_scalar(Dt[:], smax[:], smin[:], 1e-5,
                                    op0=ALU.subtract, op1=ALU.add)
            nc.vector.reciprocal(rD[:], Dt[:])
            nc.vector.tensor_scalar(s_row[:], s256[:], smin[:], rD[:],
                                    op0=ALU.subtract, op1=ALU.mult)

        # ---------------- constants (f32 iota; i32 ops are ~10x slower) --
        iota_t = sb.tile([128, 128], F32)
        nc.gpsimd.iota(iota_t[:], pattern=[[-1, 128]], base=0,
                       channel_multiplier=1,
                       allow_small_or_imprecise_dtypes=True)  # p - j
        ident_f32 = sb.tile([128, 128], F32)
        nc.vector.tensor_scalar(ident_f32[:], iota_t[:], 0, None,
                                op0=ALU.is_equal)
        ident_bf = sb.tile([128, 128], BF16)
        nc.vector.tensor_scalar(ident_bf[:], iota_t[:], 0, None,
                                op0=ALU.is_equal)
        tri_bf = sb.tile([128, 128], BF16)
        nc.vector.tensor_scalar(tri_bf[:], iota_t[:], 0.5, None,
                                op0=ALU.is_le)  # [p <= j]
        ones_bf = sb.tile([128, 128], BF16)
        nc.vector.memset(ones_bf[:], 1.0)
        ones_f = sb.tile([1, 128], F32)
        nc.vector.memset(ones_f[:], 1.0)
        diag05 = sb.tile([128, 128], F32)
        nc.vector.tensor_scalar(diag05[:], iota_t[:], 0, SIG,
                                op0=ALU.is_equal, op1=ALU.mult)
        # ACT G-Sign bias col k = 0.5 - 2k
        iota_r = sb.tile([128, K], F32)
        nc.gpsimd.iota(iota_r[:], pattern=[[-1, K]], base=0,
                       channel_multiplier=0,
                       allow_small_or_imprecise_dtypes=True)  # -j
        bias_f = sb.tile([128, K], F32)
        nc.vector.tensor_scalar(bias_f[:], iota_r[:], 2.0, 0.5,
                                op0=ALU.mult, op1=ALU.add)
        pd = [sb.tile([128, 1], F32, name=f"pd{u}") for u in range(2)]
        for u in range(2):
            nc.vector.tensor_scalar(pd[u][:], iota_t[:, 0:1],
                                    float(128 * u - 1), None, op0=ALU.add)

        # ---------------- per-chunk top-k chain --------------------------
        ApT_ps = [ps.tile([128, 512], BF16, tag="pp", name=f"ApT{u}",
                          bufs=2) for u in range(2)]
        ApT = [sb.tile([128, 512], BF16, name=f"ApTs{u}") for u in range(2)]
        for u in range(2):
            nc.gpsimd.memset(ApT[u][:], 0.0)
        cntT = [ps.tile([128, 512], F32, tag=f"ct{u}", name=f"cntT{u}")
                for u in range(2)]
        Wt = [sb.tile([128, 512], BF16, name=f"W{u}") for u in range(2)]

        def w_build(ph):
            cols = slice(256 * ph, 256 * (ph + 1))
            for u in range(2):
                nc.vector.tensor_scalar(Wt[u][:, cols], cntT[u][:, cols],
                                        pd[u][:], None, op0=ALU.add)
                nc.vector.memset(
                    ap_of(Wt[u], 256 * ph + NP,
                          [[512, 128], [128, 2], [1, 3]]), -1000.0)

        Apcs = []
        for c in range(NCH):
            pert_ps = ps.tile([128, D2], F32, tag="pt",
                              name=f"pertps{c}", bufs=2)
            nc.tensor.matmul(pert_ps[0:NP, :], ones_f[:, 0:NP], s_row[:],
                             start=True, stop=False)
            nc.tensor.matmul(pert_ps[0:NP, :], diag05[0:NP, 0:NP],
                             nz_sb[0:NP, D2 * c:D2 * (c + 1)],
                             start=False, stop=True)
            top8 = sb.tile([128, 8], F32, tag="top8", name=f"top8_{c}", bufs=2)
            nc.vector.max(top8[0:NP, :], pert_ps[0:NP, :])
            pert2 = sb.tile([128, D2], F32, tag="pert2", name=f"pert2_{c}",
                            bufs=2)
            nc.vector.match_replace(pert2[0:NP, :], top8[0:NP, :],
                                    pert_ps[0:NP, :], -1.0e30)
            top8b = sb.tile([128, 8], F32, tag="top8b", name=f"top8b_{c}",
                            bufs=2)
            nc.vector.max(top8b[0:NP, :], pert2[0:NP, :])
            bias_c = sb.tile([128, 1], F32, tag="biasc", name=f"biasc{c}",
                             bufs=2)
            nc.vector.tensor_scalar(bias_c[0:NP, :], top8b[0:NP, 7:8], -1.0,
                                    EPS, op0=ALU.mult, op1=ALU.add)
            Apc = sb.tile([128, D2], BF16, name=f"Ap{c}")
            nc.scalar.activation(Apc[0:NP, :], pert_ps[0:NP, :], ACTF.Sign,
                                 bias=bias_c[0:NP, :], scale=1.0)
            Apcs.append(Apc)
        for c in range(NCH):
            cs = slice(128 * c, 128 * (c + 1))
            for u in range(2):
                nc.tensor.transpose(ApT_ps[u][:, 128 * c:128 * c + NP],
                                    Apcs[c][0:NP, 128 * u:128 * (u + 1)],
                                    ident_bf[0:NP, 0:NP])
                (nc.vector.tensor_copy if u == 0 else
                 (lambda d, s_: nc.scalar.copy(d, s_)))(
                    ApT[u][:, 128 * c:128 * c + NP],
                    ApT_ps[u][:, 128 * c:128 * c + NP])
            nc.tensor.matmul(cntT[1][:, cs], ones_bf[:], ApT[0][:, cs],
                             start=True, stop=False)
            nc.tensor.matmul(cntT[1][:, cs], tri_bf[:], ApT[1][:, cs],
                             start=False, stop=True)
            nc.tensor.matmul(cntT[0][:, cs], tri_bf[:], ApT[0][:, cs],
                             start=True, stop=True)
            if c == 1:
                w_build(0)
            elif c == 3:
                w_build(1)

        # ---------------- G: threshold counts on W ----------------------
        # k < KD: DVE is_ge counts; k >= KD: ACT Sign-sums (2G - 512);
        # full-width [128, 512] ops (better fixed-overhead amortization)
        Gc = [sb.tile([128, K], F32, name=f"Gc{u}") for u in range(2)]
        Sa = [sb.tile([128, K], F32, name=f"Sa{u}") for u in range(2)]
        scr_v = sb.tile([128, 512], BF16, name="scr_v")
        scr_a = sb.tile([128, 512], BF16, name="scr_a")
        for u in range(2):
            for k in range(0, KD):
                nc.vector.tensor_scalar(
                    scr_v[:], Wt[u][:], 2.0 * k - 0.5, None,
                    op0=ALU.is_ge, op1=ALU.add,
                    accum_out=Gc[u][:, k:k + 1])
            for k in range(KD, K):
                nc.scalar.activation(
                    scr_a[:], Wt[u][:], ACTF.Sign,
                    bias=bias_f[:, k:k + 1], scale=1.0,
                    accum_out=Sa[u][:, k:k + 1])
        for u in range(2):
            nc.vector.tensor_scalar(Gc[u][:, KD:K], Sa[u][:, KD:K], 0.5,
                                    None, op0=ALU.mult)

        # ---------------- gct -> compact indicator ----------------------
        gct_sb = sb.tile([16, 1 + D2], F32)
        # col0 = G-form at d=-1: 0 for is_ge rows, 0.5*(-512)*INV_N for Sign
        nc.vector.tensor_scalar(gct_sb[:, 0:1], iota_t[0:16, 0:1],
                                float(KD) - 0.5, -256.0 * INV_N,
                                op0=ALU.is_ge, op1=ALU.mult)
        for u in range(2):
            gct_ps = ps.tile([16, 128], F32, tag="pp", name=f"gct{u}", bufs=2)
            nc.tensor.transpose(gct_ps[:], Gc[u][:], ident_f32[:])
            nc.vector.tensor_scalar(gct_sb[:, 1 + 128 * u:1 + 128 * (u + 1)],
                                    gct_ps[:], INV_N, None, op0=ALU.mult)
        indC = sb.tile([16, D2], F32)
        nc.vector.tensor_tensor(indC[:], gct_sb[:, 1:1 + D2],
                                gct_sb[:, 0:D2], op=ALU.subtract)
        # embed into d' = 18i + j (rims stay zero)
        indT_pad = sb.tile([16, 19 + D3], F32)
        nc.vector.memset(indT_pad[:], 0.0)
        nc.vector.tensor_copy(
            ap_of(indT_pad, 19, [[19 + D3, 16], [GE, GS], [1, GS]]),
            ap_of(indC, 0, [[D2, 16], [GS, GS], [1, GS]]))

        # ---------------- INDr: linear slices (a2-major) ----------------
        INDr = [sb.tile([CM, 64], BF16, name=f"INDr{m}") for m in range(3)]
        for m in range(3):
            tg = "pp" if m == 2 else f"ct{m}"
            ind_ps = ps.tile([CM, 64], F32, tag=tg, name=f"indps{m}",
                             bufs=2 if m == 2 else None)
            for hq in range(2):
                for wq in range(2):
                    q = 2 * hq + wq
                    s = GE * hq + wq
                    tmp = sb.tile([16, CM], F32, tag=f"iperm{q % 2}",
                                  name=f"iperm{m}_{q}", bufs=2)
                    nc.vector.tensor_copy(
                        tmp[:],
                        indT_pad[:, 19 + CM * m - s:19 + CM * (m + 1) - s])
                    nc.tensor.transpose(ind_ps[:, 16 * q:16 * (q + 1)],
                                        tmp[:], ident_f32[0:16, 0:16])
            nc.vector.tensor_copy(INDr[m][:], ind_ps[:])

        # ---------------- main matmul (bf16) + output -------------------
        act_cp = lambda d, s_: nc.scalar.copy(d, s_)
        dve_cp = nc.vector.tensor_copy
        osb = sb.tile([64, F], BF16)
        for w in range(3):
            mm = [ps.tile([64, 512], F32, tag="mm", name=f"mm{2 * w + i}",
                          bufs=2) for i in range(2)]
            for m in range(3):
                for i in range(2):
                    t = 2 * w + i
                    nc.tensor.matmul(
                        mm[i][:], INDr[m][:],
                        ap_of(B_all, m * F + 512 * t, [[3 * F, CM], [1, 512]]),
                        start=(m == 0), stop=(m == 2))
            for i in range(2):
                t = 2 * w + i
                act_cp(osb[:, 512 * t:512 * (t + 1)], mm[i][:])
                dma_s(AP(o_d, 512 * t, [[F, 64], [1, 512]]),
                      ap_of(osb, 512 * t, [[F, 64], [1, 512]]))

    nc.compile()
    return nc


def _get_nc():
    if "nc" not in _CACHE:
        _CACHE["nc"] = _build_nc()
    return _CACHE["nc"]


def _host_bx(x):
    """x (3, 512, 512) f32 -> (3*108*3072,) bf16 a2-major block gather."""
    import ml_dtypes
    xp = np.zeros((C, 576, 576), np.float32)
    xp[:, 16:528, 16:528] = x
    blocks = xp.reshape(C, GE, BLK, GE, BLK)       # (c, a, h', b, w')
    b_all = blocks.transpose(1, 3, 2, 0, 4)        # (a, b, h', c, w')
    bm = b_all.reshape(3, 6 * GE, BLK * C * BLK)   # (m, p, f)
    bm = bm.transpose(1, 0, 2)                     # (p, m, f)
    return np.ascontiguousarray(bm).reshape(-1).astype(ml_dtypes.bfloat16)


def _unscramble(o2):
    # o2 (64, 3072) rows = (hq, wq, k), cols = (h', c, w')
    return (np.asarray(o2).astype(np.float32)
              .reshape(2, 2, K, 32, C, 32)
              .transpose(2, 4, 0, 3, 1, 5)
              .reshape(K, C, PATCH, PATCH))


def _run(x_high, scores_2d, noise, trace=False):
    from concourse import bass_utils
    nc = _get_nc()
    x_high = np.ascontiguousarray(x_high, dtype=np.float32)
    scores_2d = np.ascontiguousarray(scores_2d, dtype=np.float32)
    noise = np.ascontiguousarray(noise, dtype=np.float32)
    in_maps = [
        {"bx": _host_bx(x_high[i]), "sc": scores_2d[i], "nz": noise[i]}
        for i in range(NB)
    ]
    res = bass_utils.run_bass_kernel_spmd(
        nc, in_maps, core_ids=list(range(NB)), trace=trace)
    out = np.concatenate(
        [_unscramble(res.results[i]["o"])[None] for i in range(NB)],
        axis=0).reshape(NB * K, C, PATCH, PATCH)
    return out, res


def kernel(x_high, scores_2d, noise):
    out, _ = _run(x_high, scores_2d, noise, trace=False)
    return out
